# revision 52
# baseline (speedup 1.0000x reference)
"""Trainium2 Bass kernel: 16-head MHA (B=2, S=2048, D=1024) on 8 NeuronCores.

Sharding: core c handles batch c//4 and heads 4*(c%4) .. 4*(c%4)+3
(data parallel over batch, tensor parallel over heads). Q/K/V projections
are column-sharded by head, the output projection is row-sharded; each
core emits a partial (S, D) output and the host sums the 4 partials per
batch.

Schedule (v2): the kernel is organized as one dense PE stream so the PE
HAM clock gate stays at K=8/8 (2.4 GHz). The attention phase iterates
"steps" (chunk, kb) where a chunk is (head, qc-pair); each step emits two
512-wide score matmuls into a [128,1024] PSUM region, one 1024-wide exp
on ACT (halves ACT instruction overhead vs 512-wide), and the lagged PV
pair. All other matmul work (V projection halves, mt=1 Q/K projections,
output projection) is woven between attention steps to keep the PE busy
while ACT catches up. PSUM is hand-allocated from one 8-bank tile:
banks 0-3 double-buffer the exp inputs, banks 4-7 hold the PV
accumulators by chunk parity, and weave matmuls borrow whichever parity
pair is free in their window.
"""

import sys

import numpy as np
import ml_dtypes

if "/opt/trn_rl_repo" not in sys.path:
    sys.path.insert(0, "/opt/trn_rl_repo")

B, S, D = 2, 2048, 1024
H, DK = 16, 64
NCORES = 8
HL = 4            # heads per core
DL = HL * DK      # 256 local projection dims
SCALE = 1.0 / 8.0  # 1/sqrt(DK)
LAG = 6           # PV trails scores by LAG steps
NPT = LAG + 2     # pt (exp output) ring size

_CACHE = {}


def _build_nc():
    import concourse.bass as bass  # noqa: F401
    import concourse.mybir as mybir
    from concourse import bacc, tile

    f32 = mybir.dt.float32
    bf16 = mybir.dt.bfloat16
    AF = mybir.ActivationFunctionType

    nc = bacc.Bacc(None, target_bir_lowering=False, debug=False)
    xqT = nc.declare_dram_parameter("xqT", [D, S], bf16, isOutput=False)
    xkT = nc.declare_dram_parameter("xkT", [D, S], bf16, isOutput=False)
    xvT = nc.declare_dram_parameter("xvT", [D, S], bf16, isOutput=False)
    wqT = nc.declare_dram_parameter("wqT", [D, DL], bf16, isOutput=False)
    wkT = nc.declare_dram_parameter("wkT", [D, DL], bf16, isOutput=False)
    wvT = nc.declare_dram_parameter("wvT", [D, DL], bf16, isOutput=False)
    woT = nc.declare_dram_parameter("woT", [DL, D], bf16, isOutput=False)
    y = nc.declare_dram_parameter("y", [S, D], f32, isOutput=True)

    with tile.TileContext(nc) as tc, \
         tc.tile_pool(name="singles", bufs=1) as singles, \
         tc.tile_pool(name="psum", bufs=1, space="PSUM") as pp, \
         tc.tile_pool(name="dram", bufs=1, space="DRAM") as adr:
        # ---------------- SBUF ----------------
        wq_sb = singles.tile([128, 8, DL], bf16)
        wk_sb = singles.tile([128, 8, DL], bf16)
        wv_sb = singles.tile([128, 8, DL], bf16)
        wo_sb = singles.tile([128, 2, D], bf16)
        qTm = [singles.tile([128, S], bf16, name=f"qT{m}") for m in range(2)]
        kTm = [singles.tile([128, S], bf16, name=f"kT{m}") for m in range(2)]
        atm = [singles.tile([128, S], bf16, name=f"at{m}") for m in range(2)]
        # V' per k-block: [128, mt, 2 heads x (64 v cols + ones)]
        vpst = [singles.tile([128, 2, 130], bf16, name=f"vp{st}")
                for st in range(16)]
        xq_sl = [singles.tile([128, S], bf16, name=f"xq{i}") for i in range(8)]
        xk_sl = [singles.tile([128, S], bf16, name=f"xk{i}") for i in range(8)]
        xv_sl = [singles.tile([128, S], bf16, name=f"xv{i}") for i in range(8)]
        # exp outputs (P^T), ring of NPT
        ptt = [singles.tile([128, 1024], bf16, name=f"pt{i}")
               for i in range(NPT)]
        # norm staging (rotation of 2 chunk-sized sets)
        osb = [singles.tile([65, 1024], f32, name=f"osb{i}") for i in range(2)]
        ddd = [adr.tile([1, 1024], f32, name=f"ddd{i}") for i in range(2)]
        d128 = [singles.tile([128, 8], f32, name=f"d128_{i}") for i in range(2)]
        r128 = [singles.tile([128, 8], f32, name=f"r128_{i}") for i in range(2)]
        rdd = [adr.tile([1, 1024], f32, name=f"rdd{i}") for i in range(2)]
        rec = [singles.tile([64, 1024], f32, name=f"rec{i}") for i in range(2)]
        stg = [singles.tile([64, 512], bf16, name=f"stg{i}") for i in range(4)]
        yo = [singles.tile([128, 1024], f32, name=f"yo{i}") for i in range(4)]

        # ---------------- PSUM: one 8-bank tile, hand-allocated ----------
        P = pp.tile([128, 4096], f32, name="P")

        def bank(b, w=512):
            return P[:, 512 * b: 512 * b + w]

        for st in range(16):
            nc.vector.memset(
                vpst[st].rearrange("p m (h e) -> p m h e", e=65)[:, :, :, 64:65],
                1.0)

        # ---------------- DMA issue order ----------------
        # xv first so the V projection (woven into the earliest attention
        # steps) never waits on input; xq/xk pace the q0/k0 warmup.
        for ct in range(8):
            nc.sync.dma_start(wv_sb[:, ct, :], wvT[ct * 128:(ct + 1) * 128, :])
            nc.sync.dma_start(xv_sl[ct], xvT[ct * 128:(ct + 1) * 128, :])
        for ct in range(8):
            nc.sync.dma_start(wk_sb[:, ct, :], wkT[ct * 128:(ct + 1) * 128, :])
            nc.sync.dma_start(xk_sl[ct], xkT[ct * 128:(ct + 1) * 128, :])
            nc.sync.dma_start(wq_sb[:, ct, :], wqT[ct * 128:(ct + 1) * 128, :])
            nc.sync.dma_start(xq_sl[ct], xqT[ct * 128:(ct + 1) * 128, :])
        for g in range(2):
            nc.sync.dma_start(wo_sb[:, g, :], woT[g * 128:(g + 1) * 128, :])

        # ---------------- weave closures ----------------
        def vlump(st, half, col):
            def go():
                vt = P[:, col:col + 128]
                for ct in range(8):
                    nc.tensor.matmul(
                        vt,
                        lhsT=xv_sl[ct][:, st * 128:(st + 1) * 128],
                        rhs=wv_sb[:, ct, half * 128:(half + 1) * 128],
                        start=(ct == 0), stop=(ct == 7),
                    )
                nc.vector.tensor_copy(
                    vpst[st].rearrange(
                        "p m (h e) -> p m h e", e=65)[:, half, :, 0:64],
                    vt.rearrange("p (h d) -> p h d", d=64),
                )
            return go

        def projlump(w_sb, x_sl, dst, mt, n, col):
            def go():
                acc = P[:, col:col + 512]
                for ct in range(8):
                    nc.tensor.matmul(
                        acc,
                        lhsT=w_sb[:, ct, mt * 128:(mt + 1) * 128],
                        rhs=x_sl[ct][:, n * 512:(n + 1) * 512],
                        start=(ct == 0), stop=(ct == 7),
                    )
                nc.vector.tensor_copy(dst[:, n * 512:(n + 1) * 512], acc)
            return go

        yo_i = [0]

        def y_mm(st, ec, col, g):
            # col is a 2-bank slot base; ec selects the bank. g: 0 = atm[0]
            # (K=128, starts), 1 = atm[1] (K=128, stops), "a"/"b" = the two
            # K=64 head-halves of atm[1] ("b" stops) so the tail only waits
            # on the last head's norm.
            out = P[:, col + ec * 512:col + ec * 512 + 512]
            if g in (0, 1):
                nc.tensor.matmul(
                    out,
                    lhsT=atm[g][:, st * 128:(st + 1) * 128],
                    rhs=wo_sb[:, g, ec * 512:(ec + 1) * 512],
                    start=(g == 0), stop=(g == 1),
                )
            else:
                lo = 0 if g == "a" else 64
                nc.tensor.matmul(
                    out,
                    lhsT=atm[1][lo:lo + 64, st * 128:(st + 1) * 128],
                    rhs=wo_sb[lo:lo + 64, 1, ec * 512:(ec + 1) * 512],
                    start=False, stop=(g == "b"),
                )

        def y_out(st, col, copy_engine="v"):
            # one [128,1024] copy + one DMA covers the whole y row-block
            r = yo_i[0] % 4
            yo_i[0] += 1
            if copy_engine == "v":
                nc.vector.tensor_copy(yo[r][:], P[:, col:col + 1024])
            else:
                nc.scalar.activation(yo[r][:], P[:, col:col + 1024], AF.Copy)
            nc.sync.dma_start(y[st * 128:(st + 1) * 128, :], yo[r][:])

        def ylump(st, col, copy_engine="v"):
            def go():
                for ec in range(2):
                    y_mm(st, ec, col, 0)
                for ec in range(2):
                    y_mm(st, ec, col, 1)
                y_out(st, col, copy_engine)
            return go

        # ---------------- warmup: k0 + q0(qc0,1) + V'(mt0) ----------------
        # psk[n] banks 0..3, psq[n] banks 4,5, V lumps in banks 6,7 quarters.
        # V lumps need the full xv stream, so backload them to later groups;
        # final-ct copies interleave with the closing matmuls.
        vl_sched = {3: [0, 1], 4: [2, 3], 5: [4, 5, 6],
                    6: [7, 8, 9], 7: [10, 11, 12, 13]}
        for ct in range(8):
            last = ct == 7
            for n in range(4):
                nc.tensor.matmul(
                    bank(n),
                    lhsT=wk_sb[:, ct, 0:128],
                    rhs=xk_sl[ct][:, n * 512:(n + 1) * 512],
                    start=(ct == 0), stop=last,
                )
                if last and n == 0:
                    nc.vector.tensor_copy(kTm[0][:, 0:512], bank(0))
            for n in range(2):
                nc.tensor.matmul(
                    bank(4 + n),
                    lhsT=wq_sb[:, ct, 0:128],
                    rhs=xq_sl[ct][:, n * 512:(n + 1) * 512],
                    start=(ct == 0), stop=last,
                )
                if last:
                    nc.vector.tensor_copy(
                        qTm[0][:, n * 512:(n + 1) * 512], bank(4 + n))
            if last:
                # remaining kT copies before the vlump copies queue on DVE
                # (scores need kTm chunk n by attention step 4n)
                for n in range(1, 4):
                    nc.vector.tensor_copy(
                        kTm[0][:, n * 512:(n + 1) * 512], bank(n))
            for i in vl_sched.get(ct, ()):
                vlump(i, 0, 3072 + 128 * (i % 8))()

        # weave schedule: step -> [closures]
        W = {}

        def add(step, fn):
            W.setdefault(step, []).append(fn)

        # Front-load the weave: HAM only reaches K=8/8 under sustained PE
        # density, so the early steps carry every lump whose inputs are
        # ready; the unavoidable thin stretch sits mid-stream where the
        # warm state is sticky.
        # leftover V' mt0 halves (deadlines steps 20, 21) + mt1 head start
        add(0, vlump(14, 0, 2048))
        add(2, vlump(15, 0, 2176))
        add(4, vlump(0, 1, 2304))
        # q0 qc2,3 (deadline step 32): banks 6,7
        add(2, projlump(wq_sb, xq_sl, qTm[0], 0, 2, 3072))
        add(6, projlump(wq_sb, xq_sl, qTm[0], 0, 3, 3584))
        # V-proj mt1 halves (vpst[st] mt1 needed at step 70+st)
        for i in range(1, 8):
            add(6 + 2 * i, vlump(i, 1, 3072 + 128 * (i % 8)))
        for i in range(8, 16):
            add(37 + i, vlump(i, 1, 3072 + 128 * (i % 8)))
        # q1/k1 projection: 8 full-bank lumps, deadline step 64
        qk1 = [(wk_sb, xk_sl, kTm[1]), (wq_sb, xq_sl, qTm[1])]
        for j in range(8):
            w_sb, x_sl, dst = qk1[j % 2]
            n = j // 2
            if j < 4:
                add(24 + 2 * j,
                    projlump(w_sb, x_sl, dst, 1, n, 2048 + 512 * (j % 2)))
            else:
                add(40 + 2 * (j - 4),
                    projlump(w_sb, x_sl, dst, 1, n, 3072 + 512 * (j % 2)))
        # output projection for qc 0,1 (sts 0..7): after chunk 4,5 norms.
        # 2-bank slots: banks 6,7 before chunk 7 claims them, then banks 4,5
        for j in range(8):
            if j < 2:
                add(109 + 3 * j, ylump(j, 3072, "v"))
            else:
                add(124 + 3 * (j - 2), ylump(j, 2048, "v"))
        # qc2 output rows as soon as its half-chunk norm lands
        add(142, ylump(8, 2048, "v"))
        add(145, ylump(9, 2048, "s"))

        # ---------------- attention stream ----------------
        # chunk: (mt, hh, qlo); po = 64*hh; qc pair (qlo, qlo+1).
        # The final (head 3, qc 2/3) work runs as two single-qc half-chunks
        # (steps 112..143) so qc2's norm chain overlaps the stream and only
        # qc3's gates the tail.
        chunks = [(0, 0, 0), (0, 1, 0), (0, 0, 2), (0, 1, 2),
                  (1, 0, 0), (1, 1, 0), (1, 0, 2)]
        nsteps = 16 * len(chunks)

        def ot_col(c, side):
            return (4 + 2 * (c % 2) + side) * 512

        def norm1(c):
            # copy O^T + denominators to SBUF (frees the ot banks for the
            # weave), then 1/den via a DRAM reshape so the reciprocal runs
            # 128 lanes wide, and a broadcast read back — all off the PE
            # stream; ~6-8us of DMA-queue latency hidden by the 6-step lead
            r = c % 2
            nc.vector.tensor_copy(osb[r][:], P[0:65, ot_col(c, 0):
                                               ot_col(c, 0) + 1024])
            nc.sync.dma_start(ddd[r][:], osb[r][64:65, :])
            nc.sync.dma_start(
                d128[r][:], ddd[r].rearrange("a (p j) -> (a p) j", j=8))
            nc.vector.reciprocal(r128[r][:], d128[r][:])
            nc.sync.dma_start(
                rdd[r].rearrange("a (p j) -> (a p) j", j=8), r128[r][:])
            nc.gpsimd.dma_start(out=rec[r][:],
                                in_=rdd[r].broadcast_to([64, 1024]))

        def norm2(c):
            # 6 steps later: multiply by the broadcast reciprocals (SBUF
            # only) and ship to atm
            mt, hh, qlo = chunks[c]
            po = 64 * hh
            r = c % 2
            for side in range(2):
                qc = qlo + side
                s = stg[(2 * c + side) % 4]
                nc.vector.tensor_mul(
                    s[:],
                    osb[r][0:64, side * 512:(side + 1) * 512],
                    rec[r][:, side * 512:(side + 1) * 512])
                nc.sync.dma_start(
                    atm[mt][po:po + 64, qc * 512:(qc + 1) * 512], s[:])

        def normh1(side):
            # single-qc (half-chunk) norm chain; ot in bank 6+side
            r = side
            col = (6 + side) * 512
            nc.vector.tensor_copy(osb[r][0:65, 0:512], P[0:65, col:col + 512])
            nc.sync.dma_start(ddd[r][:, 0:512], osb[r][64:65, 0:512])
            nc.sync.dma_start(
                d128[r][:, 0:4],
                ddd[r][:, 0:512].rearrange("a (p j) -> (a p) j", j=4))
            nc.vector.reciprocal(r128[r][:, 0:4], d128[r][:, 0:4])
            nc.sync.dma_start(
                rdd[r][:, 0:512].rearrange("a (p j) -> (a p) j", j=4),
                r128[r][:, 0:4])
            nc.gpsimd.dma_start(
                out=rec[r][:, 0:512],
                in_=rdd[r][:, 0:512].broadcast_to([64, 512]))

        def normh2(side):
            r = side
            qc = 2 + side
            s = stg[side]
            nc.vector.tensor_mul(s[:], osb[r][0:64, 0:512], rec[r][:, 0:512])
            nc.sync.dma_start(
                atm[1][64:128, qc * 512:(qc + 1) * 512], s[:])

        norm2_at = {}
        # half-chunks run a shorter PV lag so the final norm chains start
        # earlier (the exp pipeline is well ahead by then)
        pv_at = {}
        for c in range(len(chunks)):
            for kb in range(16):
                pv_at.setdefault(16 * c + kb + LAG, []).append((c, kb))
        for side in range(2):
            for kb in range(16):
                pv_at.setdefault(nsteps + 16 * side + kb + 3,
                                 []).append(("h", side, kb))

        def emit_step(p):
            for ent in pv_at.pop(p, ()):
                if ent[0] == "h":
                    _, side, kb = ent
                    col = (6 + side) * 512
                    nc.tensor.matmul(
                        P[0:65, col:col + 512],
                        lhsT=vpst[kb][:, 1, 65:130],
                        rhs=ptt[(nsteps + 16 * side + kb) % NPT][:, 0:512],
                        start=(kb == 0), stop=(kb == 15),
                    )
                    if kb == 15:
                        normh1(side)
                        norm2_at.setdefault(p + 6, []).append(("h", side))
                    continue
                c, kb = ent
                mt, hh, qlo = chunks[c]
                pt = ptt[(16 * c + kb) % NPT]
                for side in range(2):
                    nc.tensor.matmul(
                        P[0:65, ot_col(c, side):ot_col(c, side) + 512],
                        lhsT=vpst[kb][:, mt, hh * 65:(hh + 1) * 65],
                        rhs=pt[:, side * 512:(side + 1) * 512],
                        start=(kb == 0), stop=(kb == 15),
                    )
                if kb == 15:
                    norm1(c)
                    norm2_at.setdefault(p + 6, []).append(c)
            for ent in norm2_at.pop(p, ()):
                if isinstance(ent, tuple):
                    normh2(ent[1])
                else:
                    norm2(ent)
            # scores + exp
            if p < nsteps:
                c, kb = p // 16, p % 16
                mt, hh, qlo = chunks[c]
                po = 64 * hh
                base = (p % 2) * 1024
                for side in range(2):
                    qc = qlo + side
                    nc.tensor.matmul(
                        P[:, base + side * 512: base + side * 512 + 512],
                        lhsT=kTm[mt][po:po + 64, kb * 128:(kb + 1) * 128],
                        rhs=qTm[mt][po:po + 64, qc * 512:(qc + 1) * 512],
                        start=True, stop=True,
                    )
                nc.scalar.activation(ptt[p % NPT][:], P[:, base:base + 1024],
                                     AF.Exp, scale=SCALE)
            elif p < nsteps + 32:
                side, kb = (p - nsteps) // 16, p % 16
                base = (p % 2) * 1024
                nc.tensor.matmul(
                    P[:, base:base + 512],
                    lhsT=kTm[1][64:128, kb * 128:(kb + 1) * 128],
                    rhs=qTm[1][64:128, (2 + side) * 512:(3 + side) * 512],
                    start=True, stop=True,
                )
                nc.scalar.activation(ptt[p % NPT][:, 0:512],
                                     P[:, base:base + 512],
                                     AF.Exp, scale=SCALE)
            # weave
            for fn in W.pop(p, ()):
                fn()

        for p in range(nsteps + 32 + 3):
            emit_step(p)

        # ---------------- tail: output projection rows 10..15 ----------
        # sts 10,11 (qc2) complete immediately; qc3 rows pre-run their
        # atm[0]/qc2-independent parts while the last norm chain drains.
        def tslot(st):
            return 1024 * ((st - 8) % 4)

        for st in (10, 11):
            for ec in range(2):
                y_mm(st, ec, tslot(st), 0)
        for st in (10, 11):
            for ec in range(2):
                y_mm(st, ec, tslot(st), 1)
            y_out(st, tslot(st), "v" if st % 2 == 0 else "s")
        for st in (12, 13, 14, 15):
            for ec in range(2):
                y_mm(st, ec, tslot(st), 0)
        for key in sorted(norm2_at):
            for ent in norm2_at[key]:
                if isinstance(ent, tuple):
                    normh2(ent[1])
                else:
                    norm2(ent)
        norm2_at.clear()
        for st in (12, 13, 14, 15):
            for ec in range(2):
                y_mm(st, ec, tslot(st), 1)
            y_out(st, tslot(st), "v" if st % 2 == 0 else "s")

    nc.finalize()
    return nc


def get_nc():
    if "nc" not in _CACHE:
        _CACHE["nc"] = _build_nc()
    return _CACHE["nc"]


def make_in_maps(query, key, value, W_q, W_k, W_v, W_o):
    bf = ml_dtypes.bfloat16

    def t(a):  # contiguous transpose + bf16 cast
        return np.ascontiguousarray(np.asarray(a, np.float32).T).astype(bf)

    xq = {b: t(query[b]) for b in range(B)}
    xk = {b: t(key[b]) for b in range(B)}
    xv = {b: t(value[b]) for b in range(B)}
    W_q, W_k, W_v, W_o = (np.asarray(w, np.float32) for w in (W_q, W_k, W_v, W_o))
    wq = {g: t(W_q[g * DL:(g + 1) * DL, :]) for g in range(4)}
    wk = {g: t(W_k[g * DL:(g + 1) * DL, :]) for g in range(4)}
    wv = {g: t(W_v[g * DL:(g + 1) * DL, :]) for g in range(4)}
    wo = {g: t(W_o[:, g * DL:(g + 1) * DL]) for g in range(4)}

    in_maps = []
    for c in range(NCORES):
        b, g = divmod(c, 4)
        in_maps.append({
            "xqT": xq[b], "xkT": xk[b], "xvT": xv[b],
            "wqT": wq[g], "wkT": wk[g], "wvT": wv[g], "woT": wo[g],
        })
    return in_maps


def combine_outputs(results):
    """results: list of per-core dicts with 'y' -> full (B, S, D) output."""
    outs = [np.asarray(r["y"], np.float32) for r in results]
    return np.stack([
        outs[0] + outs[1] + outs[2] + outs[3],
        outs[4] + outs[5] + outs[6] + outs[7],
    ]).astype(np.float32)


def _exec_cached(nc, in_maps):
    """run_bass_via_pjrt with the jitted executable cached across calls."""
    import jax
    import jax.numpy as jnp  # noqa: F401
    from jax.sharding import Mesh, PartitionSpec
    from jax.experimental.shard_map import shard_map
    import concourse.mybir as mybir
    from concourse import bass2jax

    if "exec" not in _CACHE:
        bass2jax.install_neuronx_cc_hook()
        partition_name = (nc.partition_id_tensor.name
                          if nc.partition_id_tensor else None)
        in_names, out_names, out_avals = [], [], []
        for alloc in nc.m.functions[0].allocations:
            if not isinstance(alloc, mybir.MemoryLocationSet):
                continue
            name = alloc.memorylocations[0].name
            if alloc.kind == "ExternalInput":
                if name != partition_name:
                    in_names.append(name)
            elif alloc.kind == "ExternalOutput":
                out_avals.append(jax.core.ShapedArray(
                    tuple(alloc.tensor_shape), mybir.dt.np(alloc.dtype)))
                out_names.append(name)
        n_params = len(in_names)
        all_names = in_names + out_names
        if partition_name is not None:
            all_names.append(partition_name)
        donate = tuple(range(n_params, n_params + len(out_names)))

        def _body(*args):
            operands = list(args)
            if partition_name is not None:
                operands.append(bass2jax.partition_id_tensor())
            outs = bass2jax._bass_exec_p.bind(
                *operands,
                out_avals=tuple(out_avals),
                in_names=tuple(all_names),
                out_names=tuple(out_names),
                lowering_input_output_aliases=(),
                sim_require_finite=True,
                sim_require_nnan=True,
                nc=nc,
            )
            return tuple(outs)

        mesh = Mesh(np.asarray(jax.devices()[:NCORES]), ("core",))
        specs = (PartitionSpec("core"),) * (n_params + len(out_names))
        out_specs = (PartitionSpec("core"),) * len(out_names)
        _CACHE["exec"] = (
            jax.jit(shard_map(_body, mesh=mesh, in_specs=specs,
                              out_specs=out_specs, check_rep=False),
                    donate_argnums=donate, keep_unused=True),
            in_names, out_names, out_avals,
        )

    sharded, in_names, out_names, out_avals = _CACHE["exec"]
    concat_in = [
        np.concatenate([np.asarray(in_maps[c][name]) for c in range(NCORES)],
                       axis=0)
        for name in in_names
    ]
    concat_zeros = [
        np.zeros((NCORES * a.shape[0], *a.shape[1:]), a.dtype)
        for a in out_avals
    ]
    out_arrs = sharded(*concat_in, *concat_zeros)
    return [
        {name: np.asarray(out_arrs[i]).reshape(
            NCORES, *out_avals[i].shape)[c]
         for i, name in enumerate(out_names)}
        for c in range(NCORES)
    ]


def kernel(query, key, value, W_q, W_k, W_v, W_o):
    nc = get_nc()
    in_maps = make_in_maps(query, key, value, W_q, W_k, W_v, W_o)
    try:
        results = _exec_cached(nc, in_maps)
    except Exception:
        from concourse.bass_utils import run_bass_kernel_spmd
        _CACHE.pop("exec", None)
        results = run_bass_kernel_spmd(nc, in_maps, list(range(NCORES))).results
    return combine_outputs(results)
